# revision 20
# baseline (speedup 1.0000x reference)
"""Trainium2 Bass kernel for nn_CorrectSplineLinear (embedding_lookup regime).

Math: reference computes
    W[o,t,f] = sum_c interp[o,t,c] * E[c,f]        (interp = piecewise-linear in t)
    out[o,b,t] = sum_f x[b,f] * W[o,t,f]
which collapses algebraically (W2 := cv @ E precomputed on host, [128,512]
per core) to
    Z[(o,s), b] = sum_f W2[(o,s), f] * x[b, f]     (4 chunked [128,128] matmuls)
    out[o,b,t]  = sum_s hat_s(t) * Z[(o,s), b]     (hat = piecewise-linear basis)
The expansion is itself a tiny K=4 matmul per output row block, so most of it
runs on the PE: for each o, out_block[b,t] = Z_o[s,b].T @ hat[s,t].  One
[32,128] stationary (8 o-rows of Z) serves 8 matmuls: a [128,4096] basis tile
holds, for every partition base 32g and every v=o%8, a zero-padded variant
window whose hat block sits at local rows 4v..4v+3 of column block 512v
(bs[p, 512*(p%32//4)+t] = hat[p%4, t]); the moving-operand window for o then
starts at the same partition as the stationary (a PE codegen requirement) and
selects which stationary rows contribute.  The PE on this part sustains
~1.2 GHz (427ns per 512-col block), so three 2-row units are offloaded to the
otherwise-idle GpSimd engine via the direct form out = tl*dZ + Z
(tensor_scalar, per-partition scalars from a transposed ZT/dZT pair).

The kernel is memory-bound on writing the output.  The 2e-2 rel-err budget
admits fp16 (~1e-3), so the whole datapath is fp16 (fp32 PSUM accum), halving
the store stream to 4 MiB per core.  DMA throughput here is descriptor-
generation-bound (~6ns/desc per ring), so every transfer maximizes per-
partition contiguity: inputs are single full-row DMAs, and the output DRAM
layout is b-major [B, OL, I] (2-4KB contiguous per partition per store); the
host transposes to [O, B, I] and upcasts during the gather/unshard step.

Sharding: out_features O=256 split across 8 cores (32 rows each); x
replicated; each core gets its W2 = cv@E slab.
"""

import sys
from contextlib import ExitStack

import numpy as np

try:
    import concourse.bass as bass
except ImportError:  # fresh grading dir: concourse lives in the repo checkout
    sys.path.insert(0, "/opt/trn_rl_repo")
    import concourse.bass as bass

import concourse.bacc as bacc
import concourse.mybir as mybir
import concourse.tile as tile
from concourse.bass_utils import run_bass_kernel_spmd

N_CORES = 8
O, I, K, C, B = 256, 512, 3, 128, 128
OL = O // N_CORES  # 32 output rows per core
NS = K + 1  # 4 control values per output row
NZ = OL * NS  # 128 Z rows per core
F16 = mybir.dt.float16
F32 = mybir.dt.float32

# ---- packed-input column layout ([128, _PKC] fp16) ----
# 4 chunk-pairs [xT_k | w2T_k], then tl broadcast (for the GpSimd path).
_TL0 = 4 * (B + C)  # 1024
_PKC = _TL0 + I  # 1536

NU = OL // 2  # 16 drain/store units of 2 output rows
GP_UNITS = (5, 9)  # units expanded by GpSimd instead of the PE
HALF_UNITS = (0, 1, 2)  # early units drained in halves on both engines
# Output row i uses stationary slot PERM[i] (Z rows 4*PERM[i]..+4).  The
# interleave makes basis variant v = PERM[i]%8 == i//4, so early rows only
# need basis columns v0-1 and later basis quarters load off the critical
# path while the expansion already runs.
PERM = [8 * (i % 4) + i // 4 for i in range(OL)]
# store schedule: (first block, n blocks) — small head for an early HBM
# stream start, small tail so the last transfer drains quickly
_STORES = [(0, 1), (1, 1), (2, 2), (4, 2), (6, 2), (8, 4), (12, 4), (16, 4), (20, 4), (24, 4), (28, 4)]

# ---- spline geometry (input-independent, mirrors reference arithmetic) ----
_t = np.linspace(0.0, 1.0, I).astype(np.float32)
_ts = (_t * np.float32(K)).astype(np.float32)
_j = np.clip(np.floor(_ts), 0.0, float(K - 1)).astype(np.int32)
_TL = (_ts - _j.astype(np.float32)).astype(np.float32)  # [I] local coord
_b0 = int(np.searchsorted(_j, 1))  # first t index in segment 1
_b1 = int(np.searchsorted(_j, 2))  # first t index in segment 2
_SPANS = [(0, 0, _b0), (1, _b0, _b1), (2, _b1, I)]  # (segment j, t0, t1)

_cache: dict = {}


def _hat_basis() -> np.ndarray:
    """[4, I] fp32 piecewise-linear basis."""
    hat = np.zeros((NS, I), dtype=np.float32)
    hat[_j, np.arange(I)] += 1.0 - _TL
    hat[_j + 1, np.arange(I)] += _TL
    return hat


def _build_nc():
    nc = bacc.Bacc("TRN2", target_bir_lowering=False, debug=False, num_devices=N_CORES)
    pk_d = nc.dram_tensor("pk", [128, _PKC], F16, kind="ExternalInput")
    bs_d = nc.dram_tensor("bs", [128, 4096], F16, kind="ExternalInput")
    out_d = nc.dram_tensor("out", [B, OL, I], F16, kind="ExternalOutput")

    with tile.TileContext(nc) as tc, ExitStack() as ctx:
        constp = ctx.enter_context(tc.tile_pool(name="const", bufs=1))
        psump = ctx.enter_context(
            tc.tile_pool(name="psum", bufs=1, space=bass.MemorySpace.PSUM)
        )
        outp = ctx.enter_context(tc.tile_pool(name="outs", bufs=1))

        pk = constp.tile([128, _PKC], F16)
        bs = constp.tile([128, 4096], F16)
        wu = constp.tile([32, 512], F16)  # PE p-state warmup garbage
        z_sb = constp.tile([128, NZ], F16)  # Z[(o,s), b]
        zt_sb = constp.tile([128, NZ], F32)  # ZT[b, (o,s)] for the GpSimd path
        dzt_sb = constp.tile([128, NZ], F32)  # dZT[b, 4o+j] = ZT[,+1] - ZT[,.]
        outs = outp.tile([128, OL * I], F16)

        u_ps = [psump.tile([128, 1024], F32, name=f"u{i}") for i in range(4)]
        # z/zt accumulate in u_ps[3] cols 0:256 before unit 3 first needs it
        zz_ps = u_ps[3]

        # inputs: full-row slices for fat descriptors.  Only xT/w2T and
        # basis v0-1 gate the start of the expansion; tl and the later basis
        # variants land while the expansion already runs.
        # inputs: full-row slices for fat descriptors, split across both
        # rings; only xT/w2T and basis v0-1 gate the expansion start (PERM
        # makes variant v needed at expansion step 4v)
        nc.sync.dma_start(pk[:, 0:_TL0], pk_d[:, 0:_TL0])  # xT/w2T chunks
        nc.scalar.dma_start(bs[:, 0:1024], bs_d[:, 0:1024])  # v0-1
        nc.sync.dma_start(bs[:, 1024:2048], bs_d[:, 1024:2048])  # v2-3
        nc.scalar.dma_start(bs[:, 2048:3072], bs_d[:, 2048:3072])  # v4-5
        nc.sync.dma_start(pk[:, _TL0:_PKC], pk_d[:, _TL0:_PKC])  # tl
        nc.scalar.dma_start(bs[:, 3072:4096], bs_d[:, 3072:4096])  # v6-7

        tl_ap = pk[:, _TL0:_PKC]

        # PE p-state warmup: the PE runs at 1.2 GHz until ~3us into a busy
        # streak; a dummy matmul chain from right after the preamble gets it
        # to 2.4 GHz before the first real matmul, and the chain bridges the
        # wait for the pk DMA so the streak never resets.
        nc.gpsimd.memset(wu[:, :], 0.0)
        for _ in range(12):
            nc.tensor.matmul(
                u_ps[0][:, 0:512], wu[:, 0:128], wu[:, :], start=True, stop=True
            )

        # Z[slot, b] = sum_f W2[slot, f] x[b, f], chunked over f
        for k in range(4):
            base = k * (B + C)
            nc.tensor.matmul(
                zz_ps[:, 0:128],
                pk[:, base + B : base + B + C],  # lhsT [f_chunk, slot] = w2T
                pk[:, base : base + B],  # rhs  [f_chunk, b] = xT
                start=(k == 0),
                stop=(k == 3),
            )
        nc.vector.tensor_copy(z_sb[:], zz_ps[:, 0:128])  # fp32 -> fp16

        def _mm(o, pi):
            pp = PERM[o]
            g, v = pp // 8, pp % 8
            nc.tensor.matmul(
                u_ps[pi % 4][:, 512 * (o % 2) : 512 * (o % 2) + 512],
                z_sb[32 * g : 32 * g + 32, :],
                bs[32 * g : 32 * g + 32, 512 * v : 512 * v + 512],
                start=True,
                stop=True,
                tile_position=(32 * g, 0),
            )

        def _zt_chain():
            # ZT[b, slot] (swapped operands) for the GpSimd path; emitted
            # after unit 0 so it does not delay the first store
            for k in range(4):
                base = k * (B + C)
                nc.tensor.matmul(
                    zz_ps[:, 128:256],
                    pk[:, base : base + B],
                    pk[:, base + B : base + B + C],
                    start=(k == 0),
                    stop=(k == 3),
                )
            nc.vector.tensor_copy(zt_sb[:], zz_ps[:, 128:256])
            # dZT[b, i] = ZT[b, i+1] - ZT[b, i]; s=3 cols are unused garbage
            nc.gpsimd.tensor_sub(
                dzt_sb[:, 0 : NZ - 1], zt_sb[:, 1:NZ], zt_sb[:, 0 : NZ - 1]
            )

        def _direct_block(eng, o):
            # out = tl*dZ + Z with per-partition (b) scalars, one op/segment
            for jj, t0, t1 in _SPANS:
                zc = NS * PERM[o] + jj
                eng.tensor_scalar(
                    outs[:, o * I + t0 : o * I + t1],
                    tl_ap[:, t0:t1],
                    dzt_sb[:, zc : zc + 1],
                    zt_sb[:, zc : zc + 1],
                    mybir.AluOpType.mult,
                    mybir.AluOpType.add,
                )

        def _store(o0, nblk):
            nc.sync.dma_start(
                out_d[:, o0 : o0 + nblk, :],
                outs[:, o0 * I : (o0 + nblk) * I].rearrange(
                    "p (o t) -> p o t", o=nblk
                ),
            )

        store_after = {o0 + nblk - 1: (o0, nblk) for o0, nblk in _STORES}
        pi = 0  # PE-unit index for PSUM rotation
        dve_turn = False  # full-unit drains alternate Act-first from u3
        for d in range(NU):
            if d in GP_UNITS:
                _direct_block(nc.gpsimd, 2 * d)
                _direct_block(nc.gpsimd, 2 * d + 1)
            else:
                _mm(2 * d, pi)
                _mm(2 * d + 1, pi)
                if d in HALF_UNITS:
                    # split across both engines: lowest drain latency while
                    # the store stream is ramping
                    c0 = 1024 * d
                    nc.vector.tensor_copy(
                        outs[:, c0 : c0 + 512], u_ps[pi % 4][:, 0:512]
                    )
                    nc.scalar.copy(
                        outs[:, c0 + 512 : c0 + 1024], u_ps[pi % 4][:, 512:1024]
                    )
                else:
                    cols = outs[:, 1024 * d : 1024 * (d + 1)]
                    if dve_turn:
                        nc.vector.tensor_copy(cols, u_ps[pi % 4][:])
                    else:
                        nc.scalar.copy(cols, u_ps[pi % 4][:])
                    dve_turn = not dve_turn
                pi += 1
            for blk in (2 * d, 2 * d + 1):
                if blk in store_after:
                    _store(*store_after[blk])
            if d == 0:
                _zt_chain()

    nc.compile()
    return nc


def _get_nc():
    if "nc" not in _cache:
        _cache["nc"] = _build_nc()
    return _cache["nc"]


def _pack_inputs(x, control_values, expansion_matrix):
    x = np.ascontiguousarray(x, dtype=np.float32)
    cv = np.ascontiguousarray(control_values, dtype=np.float32)
    E = np.ascontiguousarray(expansion_matrix, dtype=np.float32)

    hat = _hat_basis().astype(np.float16)
    bs = np.zeros((128, 4096), dtype=np.float16)
    for p in range(128):
        v, s_ = (p % 32) // 4, p % 4
        bs[p, 512 * v : 512 * v + 512] = hat[s_]

    base = np.zeros((128, _PKC), dtype=np.float16)
    for k in range(4):
        c0 = k * (B + C)
        base[:, c0 : c0 + B] = x[:, k * 128 : (k + 1) * 128].T
    base[:, _TL0:_PKC] = _TL[None, :].astype(np.float16)

    in_maps = []
    for core in range(N_CORES):
        m = base.copy()
        # W2 rows go to permuted stationary slots: slot PERM[i] holds row i
        w2 = np.zeros((NZ, I), dtype=np.float32)
        cvc = cv[core * OL : (core + 1) * OL]  # [OL, NS, C]
        for i in range(OL):
            w2[NS * PERM[i] : NS * PERM[i] + NS] = cvc[i] @ E
        w2 = w2.astype(np.float16)
        for k in range(4):
            c0 = k * (B + C)
            m[:, c0 + B : c0 + B + C] = w2[:, k * 128 : (k + 1) * 128].T
        in_maps.append({"pk": m, "bs": bs})
    return in_maps


def _run(in_maps, trace=False):
    nc = _get_nc()
    return run_bass_kernel_spmd(
        nc, in_maps, core_ids=list(range(N_CORES)), trace=trace
    )


def _assemble(res):
    # per-core [B, OL, I] fp16 -> full [O, B, I] fp32
    out16 = np.concatenate(
        [r["out"].transpose(1, 0, 2) for r in res.results], axis=0
    )
    return np.ascontiguousarray(out16.astype(np.float32))


def kernel(x, control_points, control_values, expansion_matrix):
    in_maps = _pack_inputs(x, control_values, expansion_matrix)
    return _assemble(_run(in_maps, trace=False))


def kernel_traced(x, control_points, control_values, expansion_matrix):
    """Same as kernel() but profiles on HW; returns (out, BassKernelResults)."""
    in_maps = _pack_inputs(x, control_values, expansion_matrix)
    res = _run(in_maps, trace=True)
    return _assemble(res), res


# revision 21
# speedup vs baseline: 1.0629x; 1.0629x over previous
"""Trainium2 Bass kernel for nn_CorrectSplineLinear (embedding_lookup regime).

Math: reference computes
    W[o,t,f] = sum_c interp[o,t,c] * E[c,f]        (interp = piecewise-linear in t)
    out[o,b,t] = sum_f x[b,f] * W[o,t,f]
which collapses algebraically (W2 := cv @ E precomputed on host, [128,512]
per core) to
    Z[(o,s), b] = sum_f W2[(o,s), f] * x[b, f]     (4 chunked [128,128] matmuls)
    out[o,b,t]  = sum_s hat_s(t) * Z[(o,s), b]     (hat = piecewise-linear basis)
The expansion is itself a tiny K=4 matmul per output row block, so most of it
runs on the PE: for each o, out_block[b,t] = Z_o[s,b].T @ hat[s,t].  One
[32,128] stationary (8 o-rows of Z) serves 8 matmuls: a [128,4096] basis tile
holds, for every partition base 32g and every v=o%8, a zero-padded variant
window whose hat block sits at local rows 4v..4v+3 of column block 512v
(bs[p, 512*(p%32//4)+t] = hat[p%4, t]); the moving-operand window for o then
starts at the same partition as the stationary (a PE codegen requirement) and
selects which stationary rows contribute.  The PE on this part sustains
~1.2 GHz (427ns per 512-col block), so three 2-row units are offloaded to the
otherwise-idle GpSimd engine via the direct form out = tl*dZ + Z
(tensor_scalar, per-partition scalars from a transposed ZT/dZT pair).

The kernel is memory-bound on writing the output.  The 2e-2 rel-err budget
admits fp16 (~1e-3), so the whole datapath is fp16 (fp32 PSUM accum), halving
the store stream to 4 MiB per core.  DMA throughput here is descriptor-
generation-bound (~6ns/desc per ring), so every transfer maximizes per-
partition contiguity: inputs are single full-row DMAs, and the output DRAM
layout is b-major [B, OL, I] (2-4KB contiguous per partition per store); the
host transposes to [O, B, I] and upcasts during the gather/unshard step.

Sharding: out_features O=256 split across 8 cores (32 rows each); x
replicated; each core gets its W2 = cv@E slab.
"""

import sys
from contextlib import ExitStack

import numpy as np

try:
    import concourse.bass as bass
except ImportError:  # fresh grading dir: concourse lives in the repo checkout
    sys.path.insert(0, "/opt/trn_rl_repo")
    import concourse.bass as bass

import concourse.bacc as bacc
import concourse.mybir as mybir
import concourse.tile as tile
from concourse.bass_utils import run_bass_kernel_spmd

N_CORES = 8
O, I, K, C, B = 256, 512, 3, 128, 128
OL = O // N_CORES  # 32 output rows per core
NS = K + 1  # 4 control values per output row
NZ = OL * NS  # 128 Z rows per core
F16 = mybir.dt.float16
F32 = mybir.dt.float32

# ---- packed-input column layout ([128, _PKC] fp16) ----
# 4 chunk-pairs [xT_k | w2T_k], then tl broadcast (for the GpSimd path).
_TL0 = 4 * (B + C)  # 1024
_PKC = _TL0 + I  # 1536

NU = OL // 2  # 16 drain/store units of 2 output rows
GP_UNITS = (5, 9)  # units expanded by GpSimd instead of the PE
HALF_UNITS = (0, 1, 2)  # early units drained in halves on both engines
# Output row i uses stationary slot PERM[i] (Z rows 4*PERM[i]..+4).  The
# interleave makes basis variant v = PERM[i]%8 == i//4, so early rows only
# need basis columns v0-1 and later basis quarters load off the critical
# path while the expansion already runs.
PERM = [8 * (i % 4) + i // 4 for i in range(OL)]
# store schedule: (first block, n blocks) — small head for an early HBM
# stream start, small tail so the last transfer drains quickly
_STORES = [(0, 1), (1, 1), (2, 2), (4, 2), (6, 2), (8, 8), (16, 8), (24, 8)]

# ---- spline geometry (input-independent, mirrors reference arithmetic) ----
_t = np.linspace(0.0, 1.0, I).astype(np.float32)
_ts = (_t * np.float32(K)).astype(np.float32)
_j = np.clip(np.floor(_ts), 0.0, float(K - 1)).astype(np.int32)
_TL = (_ts - _j.astype(np.float32)).astype(np.float32)  # [I] local coord
_b0 = int(np.searchsorted(_j, 1))  # first t index in segment 1
_b1 = int(np.searchsorted(_j, 2))  # first t index in segment 2
_SPANS = [(0, 0, _b0), (1, _b0, _b1), (2, _b1, I)]  # (segment j, t0, t1)

_cache: dict = {}


def _hat_basis() -> np.ndarray:
    """[4, I] fp32 piecewise-linear basis."""
    hat = np.zeros((NS, I), dtype=np.float32)
    hat[_j, np.arange(I)] += 1.0 - _TL
    hat[_j + 1, np.arange(I)] += _TL
    return hat


def _build_nc():
    nc = bacc.Bacc("TRN2", target_bir_lowering=False, debug=False, num_devices=N_CORES)
    pk_d = nc.dram_tensor("pk", [128, _PKC], F16, kind="ExternalInput")
    bs_d = nc.dram_tensor("bs", [128, 4096], F16, kind="ExternalInput")
    out_d = nc.dram_tensor("out", [B, OL, I], F16, kind="ExternalOutput")

    with tile.TileContext(nc) as tc, ExitStack() as ctx:
        constp = ctx.enter_context(tc.tile_pool(name="const", bufs=1))
        psump = ctx.enter_context(
            tc.tile_pool(name="psum", bufs=1, space=bass.MemorySpace.PSUM)
        )
        outp = ctx.enter_context(tc.tile_pool(name="outs", bufs=1))

        pk = constp.tile([128, _PKC], F16)
        bs = constp.tile([128, 4096], F16)
        z_sb = constp.tile([128, NZ], F16)  # Z[(o,s), b]
        zt_sb = constp.tile([128, NZ], F32)  # ZT[b, (o,s)] for the GpSimd path
        dzt_sb = constp.tile([128, NZ], F32)  # dZT[b, 4o+j] = ZT[,+1] - ZT[,.]
        outs = outp.tile([128, OL * I], F16)

        u_ps = [psump.tile([128, 1024], F32, name=f"u{i}") for i in range(4)]
        # z/zt accumulate in u_ps[3] cols 0:256 before unit 3 first needs it
        zz_ps = u_ps[3]

        # inputs: full-row slices for fat descriptors.  Only xT/w2T and
        # basis v0-1 gate the start of the expansion; tl and the later basis
        # variants land while the expansion already runs.
        # inputs: full-row slices for fat descriptors, split across both
        # rings; only xT/w2T and basis v0-1 gate the expansion start (PERM
        # makes variant v needed at expansion step 4v)
        nc.sync.dma_start(pk[:, 0:_TL0], pk_d[:, 0:_TL0])  # xT/w2T chunks
        nc.scalar.dma_start(bs[:, 0:1024], bs_d[:, 0:1024])  # v0-1
        nc.sync.dma_start(bs[:, 1024:2048], bs_d[:, 1024:2048])  # v2-3
        nc.scalar.dma_start(bs[:, 2048:3072], bs_d[:, 2048:3072])  # v4-5
        nc.sync.dma_start(pk[:, _TL0:_PKC], pk_d[:, _TL0:_PKC])  # tl
        nc.scalar.dma_start(bs[:, 3072:4096], bs_d[:, 3072:4096])  # v6-7

        tl_ap = pk[:, _TL0:_PKC]

        # Z[slot, b] = sum_f W2[slot, f] x[b, f], chunked over f
        for k in range(4):
            base = k * (B + C)
            nc.tensor.matmul(
                zz_ps[:, 0:128],
                pk[:, base + B : base + B + C],  # lhsT [f_chunk, slot] = w2T
                pk[:, base : base + B],  # rhs  [f_chunk, b] = xT
                start=(k == 0),
                stop=(k == 3),
            )
        nc.vector.tensor_copy(z_sb[:], zz_ps[:, 0:128])  # fp32 -> fp16

        def _mm(o, pi):
            pp = PERM[o]
            g, v = pp // 8, pp % 8
            nc.tensor.matmul(
                u_ps[pi % 4][:, 512 * (o % 2) : 512 * (o % 2) + 512],
                z_sb[32 * g : 32 * g + 32, :],
                bs[32 * g : 32 * g + 32, 512 * v : 512 * v + 512],
                start=True,
                stop=True,
                tile_position=(32 * g, 0),
            )

        def _zt_chain():
            # ZT[b, slot] (swapped operands) for the GpSimd path; emitted
            # after unit 0 so it does not delay the first store
            for k in range(4):
                base = k * (B + C)
                nc.tensor.matmul(
                    zz_ps[:, 128:256],
                    pk[:, base : base + B],
                    pk[:, base + B : base + B + C],
                    start=(k == 0),
                    stop=(k == 3),
                )
            nc.vector.tensor_copy(zt_sb[:], zz_ps[:, 128:256])
            # dZT[b, i] = ZT[b, i+1] - ZT[b, i]; s=3 cols are unused garbage
            nc.gpsimd.tensor_sub(
                dzt_sb[:, 0 : NZ - 1], zt_sb[:, 1:NZ], zt_sb[:, 0 : NZ - 1]
            )

        def _direct_block(eng, o):
            # out = tl*dZ + Z with per-partition (b) scalars, one op/segment
            for jj, t0, t1 in _SPANS:
                zc = NS * PERM[o] + jj
                eng.tensor_scalar(
                    outs[:, o * I + t0 : o * I + t1],
                    tl_ap[:, t0:t1],
                    dzt_sb[:, zc : zc + 1],
                    zt_sb[:, zc : zc + 1],
                    mybir.AluOpType.mult,
                    mybir.AluOpType.add,
                )

        def _store(o0, nblk):
            nc.sync.dma_start(
                out_d[:, o0 : o0 + nblk, :],
                outs[:, o0 * I : (o0 + nblk) * I].rearrange(
                    "p (o t) -> p o t", o=nblk
                ),
            )

        store_after = {o0 + nblk - 1: (o0, nblk) for o0, nblk in _STORES}
        pi = 0  # PE-unit index for PSUM rotation
        dve_turn = False  # full-unit drains alternate Act-first from u3
        for d in range(NU):
            if d in GP_UNITS:
                _direct_block(nc.gpsimd, 2 * d)
                _direct_block(nc.gpsimd, 2 * d + 1)
            else:
                _mm(2 * d, pi)
                _mm(2 * d + 1, pi)
                if d in HALF_UNITS:
                    # split across both engines: lowest drain latency while
                    # the store stream is ramping
                    c0 = 1024 * d
                    nc.vector.tensor_copy(
                        outs[:, c0 : c0 + 512], u_ps[pi % 4][:, 0:512]
                    )
                    nc.scalar.copy(
                        outs[:, c0 + 512 : c0 + 1024], u_ps[pi % 4][:, 512:1024]
                    )
                else:
                    cols = outs[:, 1024 * d : 1024 * (d + 1)]
                    if dve_turn:
                        nc.vector.tensor_copy(cols, u_ps[pi % 4][:])
                    else:
                        nc.scalar.copy(cols, u_ps[pi % 4][:])
                    dve_turn = not dve_turn
                pi += 1
            for blk in (2 * d, 2 * d + 1):
                if blk in store_after:
                    _store(*store_after[blk])
            if d == 0:
                _zt_chain()

    nc.compile()
    return nc


def _get_nc():
    if "nc" not in _cache:
        _cache["nc"] = _build_nc()
    return _cache["nc"]


def _pack_inputs(x, control_values, expansion_matrix):
    x = np.ascontiguousarray(x, dtype=np.float32)
    cv = np.ascontiguousarray(control_values, dtype=np.float32)
    E = np.ascontiguousarray(expansion_matrix, dtype=np.float32)

    hat = _hat_basis().astype(np.float16)
    bs = np.zeros((128, 4096), dtype=np.float16)
    for p in range(128):
        v, s_ = (p % 32) // 4, p % 4
        bs[p, 512 * v : 512 * v + 512] = hat[s_]

    base = np.zeros((128, _PKC), dtype=np.float16)
    for k in range(4):
        c0 = k * (B + C)
        base[:, c0 : c0 + B] = x[:, k * 128 : (k + 1) * 128].T
    base[:, _TL0:_PKC] = _TL[None, :].astype(np.float16)

    in_maps = []
    for core in range(N_CORES):
        m = base.copy()
        # W2 rows go to permuted stationary slots: slot PERM[i] holds row i
        w2 = np.zeros((NZ, I), dtype=np.float32)
        cvc = cv[core * OL : (core + 1) * OL]  # [OL, NS, C]
        for i in range(OL):
            w2[NS * PERM[i] : NS * PERM[i] + NS] = cvc[i] @ E
        w2 = w2.astype(np.float16)
        for k in range(4):
            c0 = k * (B + C)
            m[:, c0 + B : c0 + B + C] = w2[:, k * 128 : (k + 1) * 128].T
        in_maps.append({"pk": m, "bs": bs})
    return in_maps


def _run(in_maps, trace=False):
    nc = _get_nc()
    return run_bass_kernel_spmd(
        nc, in_maps, core_ids=list(range(N_CORES)), trace=trace
    )


def _assemble(res):
    # per-core [B, OL, I] fp16 -> full [O, B, I] fp32
    out16 = np.concatenate(
        [r["out"].transpose(1, 0, 2) for r in res.results], axis=0
    )
    return np.ascontiguousarray(out16.astype(np.float32))


def kernel(x, control_points, control_values, expansion_matrix):
    in_maps = _pack_inputs(x, control_values, expansion_matrix)
    return _assemble(_run(in_maps, trace=False))


def kernel_traced(x, control_points, control_values, expansion_matrix):
    """Same as kernel() but profiles on HW; returns (out, BassKernelResults)."""
    in_maps = _pack_inputs(x, control_values, expansion_matrix)
    res = _run(in_maps, trace=True)
    return _assemble(res), res
